# revision 6
# baseline (speedup 1.0000x reference)
"""Trainium2 Bass kernel for EvolvedLoopLinear: out = x @ W.T + b.

Full shapes: x [4096, 4096] f32, W [4096, 4096] f32, b [4096] f32.
Sharding: 2D over 8 cores — batch split 4 ways, out_dim split 2 ways.
Per core: out_T[n, m] = sum_k W[n, k] * x[m, k] + b[n] with
  M = 1024 batch rows, N = 2048 out cols, K = 4096 contraction.
The output is computed transposed (out_dim on PSUM partitions) so the
per-partition bias rides the ACT-engine PSUM->SBUF eviction for free.
"""

import sys

for _p in ("/opt/trn_rl_repo",):
    if _p not in sys.path:
        sys.path.insert(0, _p)

import ml_dtypes
import numpy as np

import concourse.bass as bass  # noqa: F401  (registers AP machinery)
import concourse.mybir as mybir
import concourse.tile as tile
from concourse import bacc
from concourse.bass_utils import run_bass_kernel_spmd

BATCH = 4096
IN_DIM = 4096
OUT_DIM = 4096
N_CORES = 8
M_SHARD = 4  # batch split
N_SHARD = 2  # out_dim split
M = BATCH // M_SHARD  # 1024 batch rows per core
N = OUT_DIM // N_SHARD  # 2048 out cols per core
P = 128
KO = IN_DIM // P  # 32 contraction subtiles
NSUB = N // P  # 16 out-partition blocks
MT = 512  # PSUM free dim per tile
MTILES = M // MT  # 2

_CACHE: dict = {}


def _build_program(
    repeats: int = 1,
    mode: str = "bf16",
    out_engine: str = "gpsimd",
    xchunk: int = 2,
    w_split: int = 2,
    w_bufs: int = 6,
    x_engine: str = "scalar",
    w_engine: str = "sync",
    out_bufs: int = 4,
    x_bufs: int = 2,
):
    """Emit + compile the per-core SPMD program (identical on all cores).

    repeats > 1 wraps the whole body in a dynamic For_i loop — used only
    for steady-state timing (the body is idempotent)."""
    nc = bacc.Bacc("TRN2", target_bir_lowering=False, debug=False, num_devices=N_CORES)
    dt = {
        "bf16": mybir.dt.bfloat16,
        "fp32r": mybir.dt.float32r,
        "fp32": mybir.dt.float32,
    }[mode]
    xt = nc.declare_dram_parameter("xt", [P, KO, M], dt, isOutput=False)
    wt = nc.declare_dram_parameter("wt", [P, NSUB, KO, P], dt, isOutput=False)
    bs = nc.declare_dram_parameter("bs", [P, NSUB], mybir.dt.float32, isOutput=False)
    ot = nc.declare_dram_parameter("ot", [P, NSUB, M], mybir.dt.float32, isOutput=True)

    with tile.TileContext(nc) as tc:
        with (
            tc.tile_pool(name="xres", bufs=1) as xres_pool,
            tc.tile_pool(name="wblk", bufs=w_bufs) as w_pool,
            tc.tile_pool(name="psum", bufs=8, space="PSUM") as psum_pool,
            tc.tile_pool(name="outp", bufs=out_bufs) as out_pool,
            tc.tile_pool(name="bias", bufs=1) as b_pool,
        ):

            def body(_iv=None):
                bias_sb = b_pool.tile([P, NSUB], mybir.dt.float32)
                nc.sync.dma_start(bias_sb[:], bs[:])

                # x shard stays SBUF-resident (16 MB); load in KO-chunks
                # so compute can start before the whole shard lands.
                xres = xres_pool.tile([P, KO, M], dt)
                XCHUNK = xchunk
                x_dma = getattr(nc, x_engine)
                for kc in range(0, KO, XCHUNK):
                    x_dma.dma_start(
                        xres[:, kc : kc + XCHUNK], xt[:, kc : kc + XCHUNK]
                    )

                # W streams in half-K blocks (8 KB/partition) with deep
                # buffering so the next block's DMA hides under compute.
                KHALF = KO // w_split
                for ns in range(NSUB):
                    whs = [
                        w_pool.tile([P, KHALF, P], dt, name=f"wh{i}", tag="wh")
                        for i in range(w_split)
                    ]
                    w_dma = getattr(nc, w_engine)
                    for i in range(w_split):
                        w_dma.dma_start(
                            whs[i][:], wt[:, ns, i * KHALF : (i + 1) * KHALF]
                        )
                    pts = [
                        psum_pool.tile([P, MT], mybir.dt.float32, name=f"pt{i}", tag="pt")
                        for i in range(MTILES)
                    ]
                    # ko outer / mt inner: consecutive matmuls share the
                    # stationary W block, halving LDWEIGHTS traffic.
                    for ko in range(KO):
                        for mt in range(MTILES):
                            nc.tensor.matmul(
                                pts[mt][:],
                                whs[ko // KHALF][:, ko % KHALF],
                                xres[:, ko, mt * MT : (mt + 1) * MT],
                                start=(ko == 0),
                                stop=(ko == KO - 1),
                            )
                    for mt in range(MTILES):
                        ot_sb = out_pool.tile([P, MT], mybir.dt.float32)
                        nc.scalar.add(ot_sb[:], pts[mt][:], bias_sb[:, ns : ns + 1])
                        out_dma = nc.gpsimd if out_engine == "gpsimd" else nc.sync
                        out_dma.dma_start(
                            ot[:, ns, mt * MT : (mt + 1) * MT], ot_sb[:]
                        )

            if repeats == 1:
                body()
            else:
                with tc.For_i(0, repeats, 1) as iv:
                    body(iv)

    nc.compile()
    return nc


def _shard_inputs(x: np.ndarray, W: np.ndarray, b: np.ndarray, mode: str = "bf16"):
    """Host-side shard + retile into the DMA-friendly layouts.

    For bf16 mode the matmul operands are cast host-side (RTN); the bias
    and PSUM accumulation stay fp32, so the only precision loss is the
    input rounding (measured rel err ~2e-3 vs the 2e-2 gate).
    """
    np_dt = ml_dtypes.bfloat16 if mode == "bf16" else np.float32
    in_maps = []
    xt_cache = {}
    wt_cache = {}
    bs_cache = {}
    for c in range(N_CORES):
        q, h = divmod(c, N_SHARD)
        if q not in xt_cache:
            xs = x[q * M : (q + 1) * M]  # [M, IN]
            xt_cache[q] = np.ascontiguousarray(
                xs.reshape(M, KO, P).transpose(2, 1, 0).astype(np_dt)
            )
        if h not in wt_cache:
            Ws = W[h * N : (h + 1) * N]  # [N, IN]
            wt_cache[h] = np.ascontiguousarray(
                Ws.reshape(NSUB, P, KO, P).transpose(3, 0, 2, 1).astype(np_dt)
            )
            bs_cache[h] = np.ascontiguousarray(
                b[h * N : (h + 1) * N].reshape(NSUB, P).T
            )
        in_maps.append({"xt": xt_cache[q], "wt": wt_cache[h], "bs": bs_cache[h]})
    return in_maps


def _assemble(results) -> np.ndarray:
    out = np.empty((BATCH, OUT_DIM), dtype=np.float32)
    for c in range(N_CORES):
        q, h = divmod(c, N_SHARD)
        ot = results[c]["ot"]  # [P, NSUB, M]
        block = ot.transpose(2, 1, 0).reshape(M, N)
        out[q * M : (q + 1) * M, h * N : (h + 1) * N] = block
    return out


def kernel(x: np.ndarray, W: np.ndarray, b: np.ndarray) -> np.ndarray:
    x = np.asarray(x, dtype=np.float32)
    W = np.asarray(W, dtype=np.float32)
    b = np.asarray(b, dtype=np.float32)
    assert x.shape == (BATCH, IN_DIM) and W.shape == (OUT_DIM, IN_DIM)

    if "nc" not in _CACHE:
        _CACHE["nc"] = _build_program()
    nc = _CACHE["nc"]

    in_maps = _shard_inputs(x, W, b)
    res = run_bass_kernel_spmd(nc, in_maps, list(range(N_CORES)))
    return _assemble(res.results)


if __name__ == "__main__":
    rng = np.random.default_rng(0)
    x = rng.standard_normal((BATCH, IN_DIM), dtype=np.float32)
    W = rng.uniform(-1 / 64, 1 / 64, size=(OUT_DIM, IN_DIM)).astype(np.float32)
    b = rng.uniform(-1 / 64, 1 / 64, size=(OUT_DIM,)).astype(np.float32)
    got = kernel(x, W, b)
    exp = x @ W.T + b
    scale = np.abs(exp).max()
    print("absmax err:", np.abs(got - exp).max(), "scale:", scale)



# revision 30
# speedup vs baseline: 10.8825x; 10.8825x over previous
"""Trainium2 Bass kernel for EvolvedLoopLinear: out = x @ W.T + b.

Full shapes: x [4096, 4096] f32, W [4096, 4096] f32, b [4096] f32.
Sharding: 2D over 8 cores — batch split 4 ways, out_dim split 2 ways.
Per core: out_T[n, m] = sum_k W[n, k] * x[m, k] + b[n] with
  M = 1024 batch rows, N = 2048 out cols, K = 4096 contraction.
The output is computed transposed (out_dim on PSUM partitions) so the
per-partition bias rides the ACT-engine PSUM->SBUF eviction for free.

Design (vs the fp32r original):
- Matmul operands are cast to bf16 host-side (RTN); PSUM accumulation
  and the bias add stay fp32. Measured rel err 2.1e-3 vs the 2e-2
  gate. This halves DMA traffic and SBUF footprint and streams the PE
  at 1 cycle/row.
- Loop order is mt-outer/ko-inner (production composable_matmul
  order): the PSUM target is constant across the K loop and each MM's
  fresh weights prefetch into the PE background weight buffer.
- The SBUF-resident x shard (64 KB/partition in bf16) is
  double-buffered so the next repeat's x DMA overlaps tail matmuls.
- Measured on HW: per-iteration time tracks total streamed PE columns
  only (~1.8 Gcol/s/core sustained, chip power/thermal limited with
  all 8 cores busy; single-core runs ~17% faster). Schedule is clean:
  tile-sim shows PE at 100% occupancy. PSUM matmul outputs must stay
  within one 2 KB bank (512 fp32) — a [128, 1024] fp32 output
  compiles but crashes the exec unit.
"""

import sys

for _p in ("/opt/trn_rl_repo",):
    if _p not in sys.path:
        sys.path.insert(0, _p)

import ml_dtypes
import numpy as np

import concourse.bass as bass  # noqa: F401  (registers AP machinery)
import concourse.mybir as mybir
import concourse.tile as tile
from concourse import bacc
from concourse.bass_utils import run_bass_kernel_spmd

BATCH = 4096
IN_DIM = 4096
OUT_DIM = 4096
N_CORES = 8
M_SHARD = 4  # batch split
N_SHARD = 2  # out_dim split
M = BATCH // M_SHARD  # 1024 batch rows per core
N = OUT_DIM // N_SHARD  # 2048 out cols per core
P = 128
KO = IN_DIM // P  # 32 contraction subtiles
NSUB = N // P  # 16 out-partition blocks
MT = 512  # PSUM free dim per tile
MTILES = M // MT  # 2
W_FP8_SCALE = 64.0  # host-side W scale for fp8e3 mode

_CACHE: dict = {}


def _build_program(
    repeats: int = 1,
    mode: str = "bf16",
    out_engine: str = "gpsimd",
    xchunk: int = 2,
    w_split: int = 2,
    w_bufs: int = 6,
    x_engine: str = "scalar",
    w_engine: str = "sync",
    out_bufs: int = 4,
    x_bufs: int | None = None,
    order: str = "mt_ko",
    fake_w: bool = False,
    fake_w_same: bool = False,
    mt_size: int = MT,
    psum_bufs: int = 8,
    passes: int = 1,
    out_bf16: bool = False,
):
    """Emit + compile the per-core SPMD program (identical on all cores).

    repeats > 1 wraps the whole body in a dynamic For_i loop — used only
    for steady-state timing (the body is idempotent)."""
    nc = bacc.Bacc("TRN2", target_bir_lowering=False, debug=False, num_devices=N_CORES)
    dt = {
        "bf16": mybir.dt.bfloat16,
        "fp8e3": mybir.dt.float8e3,
        "fp32r": mybir.dt.float32r,
        "fp32": mybir.dt.float32,
    }[mode]
    # fp8e3 (e3m4) needs W pre-scaled by 64 host-side so the uniform
    # (-1/64, 1/64) weights land in the normal range; the PSUM result is
    # 64x too big and is rescaled during the ACT bias-add eviction.
    out_scale = 1.0 / W_FP8_SCALE if mode == "fp8e3" else 1.0
    if x_bufs is None:
        # Double-buffer the SBUF-resident x shard (64 KB/partition in
        # bf16) so the next repeat's x DMA overlaps the tail matmuls;
        # fp32 x is 128 KB/partition, too big to double-buffer.
        x_bufs = 2 if mode in ("bf16", "fp8e3") else 1
    xt = nc.declare_dram_parameter("xt", [P, KO, M], dt, isOutput=False)
    wt = nc.declare_dram_parameter("wt", [P, NSUB, KO, P], dt, isOutput=False)
    bs = nc.declare_dram_parameter("bs", [P, NSUB], mybir.dt.float32, isOutput=False)
    out_dt = mybir.dt.bfloat16 if out_bf16 else mybir.dt.float32
    ot = nc.declare_dram_parameter("ot", [P, NSUB, M], out_dt, isOutput=True)

    with tile.TileContext(nc) as tc:
        with (
            tc.tile_pool(name="xres", bufs=x_bufs) as xres_pool,
            tc.tile_pool(name="wblk", bufs=w_bufs) as w_pool,
            tc.tile_pool(name="psum", bufs=psum_bufs, space="PSUM") as psum_pool,
            tc.tile_pool(name="outp", bufs=out_bufs) as out_pool,
            tc.tile_pool(name="bias", bufs=1) as b_pool,
        ):

            def body(_iv=None):
                bias_sb = b_pool.tile([P, NSUB], mybir.dt.float32)
                nc.sync.dma_start(bias_sb[:], bs[:])

                # x shard stays SBUF-resident (16 MB); load in KO-chunks
                # so compute can start before the whole shard lands.
                xres = xres_pool.tile([P, KO, M], dt)
                XCHUNK = xchunk
                x_dma = getattr(nc, x_engine)
                for kc in range(0, KO, XCHUNK):
                    x_dma.dma_start(
                        xres[:, kc : kc + XCHUNK], xt[:, kc : kc + XCHUNK]
                    )

                # W streams in half-K blocks (8 KB/partition) with deep
                # buffering so the next block's DMA hides under compute.
                MTS = mt_size
                NMT = M // MTS
                KHALF = KO // w_split
                w_dma = getattr(nc, w_engine)
                if fake_w or fake_w_same:
                    # Diagnostic: one W block loaded once, reused for all
                    # ns — wrong numerics, isolates PE stream from W DMA.
                    fwhs = [
                        w_pool.tile([P, KHALF, P], dt, name=f"fwh{i}", tag="wh")
                        for i in range(w_split)
                    ]
                    for i in range(w_split):
                        w_dma.dma_start(
                            fwhs[i][:], wt[:, 0, i * KHALF : (i + 1) * KHALF]
                        )
                for ns in [n for _ in range(passes) for n in range(NSUB)]:
                    if fake_w or fake_w_same:
                        whs = fwhs
                    else:
                        whs = [
                            w_pool.tile([P, KHALF, P], dt, name=f"wh{i}", tag="wh")
                            for i in range(w_split)
                        ]
                        for i in range(w_split):
                            w_dma.dma_start(
                                whs[i][:], wt[:, ns, i * KHALF : (i + 1) * KHALF]
                            )
                    pts = [
                        psum_pool.tile([P, MTS], mybir.dt.float32, name=f"pt{i}", tag="pt")
                        for i in range(NMT)
                    ]

                    def lhsT_for(ko):
                        if fake_w_same:
                            # Identical stationary operand for every MM —
                            # probes whether weight (re)loading is what
                            # serializes the MM stream on HW.
                            return whs[0][:, 0]
                        return whs[ko // KHALF][:, ko % KHALF]

                    if order == "ko_mt":
                        # ko outer / mt inner: consecutive matmuls share the
                        # stationary W block, halving LDWEIGHTS traffic —
                        # but cycles PSUM banks every MM.
                        for ko in range(KO):
                            for mt in range(NMT):
                                nc.tensor.matmul(
                                    pts[mt][:],
                                    lhsT_for(ko),
                                    xres[:, ko, mt * MTS : (mt + 1) * MTS],
                                    start=(ko == 0),
                                    stop=(ko == KO - 1),
                                )
                    else:
                        # mt outer / ko inner (production composable_matmul
                        # order): PSUM target constant across the K loop;
                        # each MM's fresh weights prefetch into the PE's
                        # background weight buffer under the running MM.
                        for mt in range(NMT):
                            for ko in range(KO):
                                nc.tensor.matmul(
                                    pts[mt][:],
                                    lhsT_for(ko),
                                    xres[:, ko, mt * MTS : (mt + 1) * MTS],
                                    start=(ko == 0),
                                    stop=(ko == KO - 1),
                                )
                    for mt in range(NMT):
                        ot_sb = out_pool.tile([P, MTS], out_dt)
                        nc.scalar.activation(
                            ot_sb[:],
                            pts[mt][:],
                            mybir.ActivationFunctionType.Identity,
                            bias=bias_sb[:, ns : ns + 1],
                            scale=out_scale,
                        )
                        out_dma = nc.gpsimd if out_engine == "gpsimd" else nc.sync
                        out_dma.dma_start(
                            ot[:, ns, mt * MTS : (mt + 1) * MTS], ot_sb[:]
                        )

            if repeats == 1:
                body()
            else:
                with tc.For_i(0, repeats, 1) as iv:
                    body(iv)

    nc.compile()
    return nc


def _shard_inputs(x: np.ndarray, W: np.ndarray, b: np.ndarray, mode: str = "bf16"):
    """Host-side shard + retile into the DMA-friendly layouts.

    For bf16 mode the matmul operands are cast host-side (RTN); the bias
    and PSUM accumulation stay fp32, so the only precision loss is the
    input rounding (measured rel err ~2e-3 vs the 2e-2 gate).
    """
    np_dt = {
        "bf16": ml_dtypes.bfloat16,
        "fp8e3": ml_dtypes.float8_e3m4,
        "fp32r": np.float32,
        "fp32": np.float32,
    }[mode]
    w_scale = W_FP8_SCALE if mode == "fp8e3" else 1.0
    in_maps = []
    xt_cache = {}
    wt_cache = {}
    bs_cache = {}
    for c in range(N_CORES):
        q, h = divmod(c, N_SHARD)
        if q not in xt_cache:
            xs = x[q * M : (q + 1) * M]  # [M, IN]
            xt_cache[q] = np.ascontiguousarray(
                xs.reshape(M, KO, P).transpose(2, 1, 0).astype(np_dt)
            )
        if h not in wt_cache:
            Ws = W[h * N : (h + 1) * N] * w_scale  # [N, IN]
            wt_cache[h] = np.ascontiguousarray(
                Ws.reshape(NSUB, P, KO, P).transpose(3, 0, 2, 1).astype(np_dt)
            )
            bs_cache[h] = np.ascontiguousarray(
                b[h * N : (h + 1) * N].reshape(NSUB, P).T
            )
        in_maps.append({"xt": xt_cache[q], "wt": wt_cache[h], "bs": bs_cache[h]})
    return in_maps


def _assemble(results) -> np.ndarray:
    out = np.empty((BATCH, OUT_DIM), dtype=np.float32)
    for c in range(N_CORES):
        q, h = divmod(c, N_SHARD)
        ot = results[c]["ot"]  # [P, NSUB, M]
        block = ot.transpose(2, 1, 0).reshape(M, N)
        out[q * M : (q + 1) * M, h * N : (h + 1) * N] = block
    return out


def kernel(x: np.ndarray, W: np.ndarray, b: np.ndarray) -> np.ndarray:
    x = np.asarray(x, dtype=np.float32)
    W = np.asarray(W, dtype=np.float32)
    b = np.asarray(b, dtype=np.float32)
    assert x.shape == (BATCH, IN_DIM) and W.shape == (OUT_DIM, IN_DIM)

    if "nc" not in _CACHE:
        _CACHE["nc"] = _build_program()
    nc = _CACHE["nc"]

    in_maps = _shard_inputs(x, W, b)
    res = run_bass_kernel_spmd(nc, in_maps, list(range(N_CORES)))
    return _assemble(res.results)


if __name__ == "__main__":
    rng = np.random.default_rng(0)
    x = rng.standard_normal((BATCH, IN_DIM), dtype=np.float32)
    W = rng.uniform(-1 / 64, 1 / 64, size=(OUT_DIM, IN_DIM)).astype(np.float32)
    b = rng.uniform(-1 / 64, 1 / 64, size=(OUT_DIM,)).astype(np.float32)
    got = kernel(x, W, b)
    exp = x @ W.T + b
    scale = np.abs(exp).max()
    print("absmax err:", np.abs(got - exp).max(), "scale:", scale)

